# revision 17
# baseline (speedup 1.0000x reference)
"""Butterfly block-sparse linear kernel for Trainium2 (8 NeuronCores, SPMD).

Computes: y = blockdiag_butterfly(x, factorL, factorR) + bias
  x:(4,2048,4096) f32, factorL/factorR:(8,512,512) f32, bias:(4096,) f32

Math (reference):
  out1[b,k,q] = sum_p x[b, 512k+p] * factorL[k,q,p]      (8 blocks of 512x512)
  z[b,l,r]    = out1_flat[b, 8r+l]                        (butterfly permute)
  out2[b,l,s] = sum_r z[b,l,r] * factorR[l,s,r]
  y[b, 8s+l]  = out2[b,l,s] + bias[8s+l]

Strategy: data-parallel over the 8192 tokens (1024 tokens/core), factors
replicated, all on-chip data bf16 (f32 PSUM accumulation).

The butterfly permute is absorbed entirely into the host-side weight
layouts so the device does NO data movement for it:
  - Stage-1 output channels are ordered q' = 128*((q%8)//2) + 64*((q%8)%2)
    + q//8, so PSUM tile (k,qc) holds, on partitions 0:64, the stage-2 rows
    r=64k..64k+63 of block l=2qc and, on partitions 64:128, the same rows
    of block l=2qc+1.  Each PSUM tile is evicted FULL-WIDTH (one engine op,
    f32->bf16) straight into the stage-2 input tile z[qc] at column block
    (b,k) -- no staging, no SBUF->SBUF permute DMAs, no barrier.
  - Stage 2 contracts the 8 k-chunks of 64 rows with row-tiled matmul
    PAIRS: partitions 0:64 (block 2qc) and 64:128 (block 2qc+1) run
    concurrently in different row-groups of the PE array (tile_position
    auto-derived from base partitions), so the pair costs one 512-column
    slot, preserving the K=128 PE pace.

Schedule: S1(b0) S2(b0) S1(b1) S2(b1), no explicit waits.  ~8 dummy
matmuls on a scratch tile run during the DMA cold start so the PE HAM
un-throttles (1.2->2.4 GHz) before real data lands.  Evictions alternate
between DVE and ACT; stage-2 eviction fuses the bias add and bf16
downcast.  Weights stream on the gpsimd queue, x on sync/scalar queues in
pc-sized first chunks so the first matmul starts as early as possible;
stores stream out per 2-sc-block on sync/gpsimd.

Output leaves the device in device order (rows b,l,sc,j; cols t); the host
does the final (cheap) gather back to token-major f32.
"""

import os
import numpy as np
from contextlib import ExitStack

NCORES = 8
TOK = 8192
TPC = TOK // NCORES          # tokens per core
T = 512                      # tokens per on-chip batch
NB = TPC // T                # 2 batches
NDUMMY = 10                  # PE warm-up matmuls on scratch data

_CACHE = {}
LAST_RESULT = None


def _build_program():
    import concourse.bacc as bacc
    import concourse.tile as tile
    import concourse.mybir as mybir

    F32 = mybir.dt.float32
    BF16 = mybir.dt.bfloat16

    nc = bacc.Bacc("TRN2", target_bir_lowering=False, debug=False)
    # x rows = (b, k, pp), cols = (pc, t): per (b,k) one [128,2048] tile
    x = nc.dram_tensor("x", [NB * 8 * 128, 2048], BF16, kind="ExternalInput").ap()
    # w1 cols = (k, pc, qc, j): j = output q' within 128-block
    w1 = nc.dram_tensor("w1", [128, 16384], BF16, kind="ExternalInput").ap()
    # w2 cols = (qc, k, sc, j): partitions 0:64 block l=2qc rows 64k+m,
    # partitions 64:128 block l=2qc+1
    w2 = nc.dram_tensor("w2", [128, 16384], BF16, kind="ExternalInput").ap()
    bias = nc.dram_tensor("bias", [128, 32], F32, kind="ExternalInput").ap()
    # out rows = (b, l, j), cols = (sc, t): 2KB-contiguous per partition so
    # store DMA packets are large (device order; host unscrambles)
    out = nc.dram_tensor("out", [NB * 8 * 128, 4 * T], BF16, kind="ExternalOutput").ap()

    x_r = x.rearrange("(b k p) c -> b k p c", b=NB, k=8)
    out_r = out.rearrange("(g p) c -> g p c", p=128)

    with tile.TileContext(nc) as tc, ExitStack() as ctx:
        wpool = ctx.enter_context(tc.tile_pool(name="w", bufs=1))
        xpool = ctx.enter_context(tc.tile_pool(name="x", bufs=10))
        zpool = ctx.enter_context(tc.tile_pool(name="z", bufs=1))
        opool = ctx.enter_context(tc.tile_pool(name="o", bufs=4))
        pspool = ctx.enter_context(tc.tile_pool(name="ps", bufs=8, space="PSUM"))

        bt = wpool.tile([128, 32], F32, tag="bias")
        w1t = wpool.tile([128, 16384], BF16, tag="w1")
        w2t = wpool.tile([128, 16384], BF16, tag="w2")
        scratch = wpool.tile([128, 512], BF16, tag="scratch")
        # z[b][qc]: [128, (k, t)] -- partitions 0:64 l=2qc, 64:128 l=2qc+1
        zts = [
            [
                zpool.tile([128, 8 * T], BF16, name=f"z_{b}_{qc}", tag=f"z_{b}_{qc}")
                for qc in range(4)
            ]
            for b in range(NB)
        ]

        def _evcopy(eng, dst, src):
            if eng is nc.scalar:
                eng.activation(dst, src, mybir.ActivationFunctionType.Identity)
            else:
                eng.tensor_copy(dst, src)

        xloads = {}

        def load_x(eng, b, k, split=False, eng2=None):
            xt = xpool.tile([128, 2048], BF16, tag="xt")
            if split:
                # first block only: small pc0 chunk so the first matmul can
                # start ASAP, remainder as one DMA (bigger packets)
                eng.dma_start(xt[:, 0:T], x_r[b, k, :, 0:T])
                eng.dma_start(xt[:, T:2048], x_r[b, k, :, T:2048])
            elif eng2 is not None:
                # halves on two queues for an early deadline
                eng.dma_start(xt[:, 0:1024], x_r[b, k, :, 0:1024])
                eng2.dma_start(xt[:, 1024:2048], x_r[b, k, :, 1024:2048])
            else:
                eng.dma_start(xt[:], x_r[b, k])
            xloads[(b, k)] = xt

        def load_w(eng, wt, src, j, nchunk=1):
            # load 512-col chunk group [j*512, (j+nchunk)*512)
            eng.dma_start(
                wt[:, j * 512 : (j + nchunk) * 512], src[:, j * 512 : (j + nchunk) * 512]
            )

        # ---------------- stage 1 ----------------
        def s1(b, k):
            xt = xloads.pop((b, k))
            p1s = [pspool.tile([128, T], F32, name="p1", tag="ps") for _ in range(4)]
            for pc in range(4):
                for qc in range(4):
                    col = k * 2048 + pc * 512 + qc * 128
                    nc.tensor.matmul(
                        p1s[qc][:],
                        w1t[:, col : col + 128],
                        xt[:, pc * T : (pc + 1) * T],
                        start=(pc == 0),
                        stop=(pc == 3),
                    )
            for qc in range(4):
                eng = nc.vector if (qc + k) % 2 == 0 else nc.scalar
                _evcopy(eng, zts[b][qc][:, k * T : (k + 1) * T], p1s[qc][:])

        # ---------------- stage 2 ----------------
        def s2(b, qc):
            z = zts[b][qc]
            ot_e = opool.tile([128, 4 * T], BF16, tag="ot")
            ot_o = opool.tile([128, 4 * T], BF16, tag="ot")
            l_e, l_o = 2 * qc, 2 * qc + 1
            for sc in range(4):
                p2e = pspool.tile([128, T], F32, tag="ps")
                p2o = pspool.tile([128, T], F32, tag="ps")
                for k in range(8):
                    col = qc * 4096 + k * 512 + sc * 128
                    nc.tensor.matmul(
                        p2e[:],
                        w2t[0:64, col : col + 128],
                        z[0:64, k * T : (k + 1) * T],
                        start=(k == 0),
                        stop=(k == 7),
                    )
                    nc.tensor.matmul(
                        p2o[:],
                        w2t[64:128, col : col + 128],
                        z[64:128, k * T : (k + 1) * T],
                        start=(k == 0),
                        stop=(k == 7),
                    )
                # evictions with fused bias add + bf16 downcast
                be = bt[:, l_e * 4 + sc : l_e * 4 + sc + 1]
                bo = bt[:, l_o * 4 + sc : l_o * 4 + sc + 1]
                ee = nc.vector if sc % 2 == 0 else nc.scalar
                eo = nc.scalar if sc % 2 == 0 else nc.vector
                if ee is nc.vector:
                    nc.vector.tensor_scalar(
                        out=ot_e[:, sc * T : (sc + 1) * T], in0=p2e[:],
                        scalar1=be, scalar2=None, op0=mybir.AluOpType.add,
                    )
                else:
                    nc.scalar.activation(
                        ot_e[:, sc * T : (sc + 1) * T], p2e[:],
                        mybir.ActivationFunctionType.Identity, bias=be,
                    )
                if eo is nc.vector:
                    nc.vector.tensor_scalar(
                        out=ot_o[:, sc * T : (sc + 1) * T], in0=p2o[:],
                        scalar1=bo, scalar2=None, op0=mybir.AluOpType.add,
                    )
                else:
                    nc.scalar.activation(
                        ot_o[:, sc * T : (sc + 1) * T], p2o[:],
                        mybir.ActivationFunctionType.Identity, bias=bo,
                    )
                # stream stores out per 2-sc half (2KB-contiguous spans) so
                # the tail drains early; sync/scalar (HWDGE) so the gpsimd
                # queue is long-drained before the kernel epilogue.
                ge = b * 8 + l_e
                go = b * 8 + l_o
                if b == 1 and qc == 3 and sc >= 2:
                    # very last tiles: store per-sc so the tail drain starts
                    # as early as possible
                    nc.sync.dma_start(
                        out_r[ge, :, sc * T : (sc + 1) * T],
                        ot_e[:, sc * T : (sc + 1) * T],
                    )
                    nc.scalar.dma_start(
                        out_r[go, :, sc * T : (sc + 1) * T],
                        ot_o[:, sc * T : (sc + 1) * T],
                    )
                elif sc == 1 or sc == 3:
                    h = sc // 2  # 0 or 1
                    nc.sync.dma_start(
                        out_r[ge, :, 2 * h * T : (2 * h + 2) * T],
                        ot_e[:, 2 * h * T : (2 * h + 2) * T],
                    )
                    nc.scalar.dma_start(
                        out_r[go, :, 2 * h * T : (2 * h + 2) * T],
                        ot_o[:, 2 * h * T : (2 * h + 2) * T],
                    )

        # ---------------- schedule ----------------
        # PE warm-up: dummy matmuls on scratch data during the DMA cold
        # start so HAM un-throttles before real data lands.
        nc.vector.memset(scratch[:], 0)
        warm = pspool.tile([128, T], F32, name="warm", tag="ps")
        for _ in range(NDUMMY):
            nc.tensor.matmul(warm[:], scratch[:, 0:128], scratch[:], start=True, stop=True)

        # first chunks on the two HWDGE queues in small pieces so the first
        # matmul starts ASAP; rest of w1/w2 streams on gpsimd (slow cold
        # start, hidden behind the first chunks) in 8KB-span chunks.
        load_x(nc.sync, 0, 0, split=True)
        nc.scalar.dma_start(w1t[:, 0:512], w1[:, 0:512])          # k0 pc0
        nc.scalar.dma_start(w1t[:, 512:2048], w1[:, 512:2048])    # k0 pc1-3
        load_x(nc.sync, 0, 1, eng2=nc.scalar)  # halves on both HWDGE queues
        load_w(nc.gpsimd, w1t, w1, 4, nchunk=4)                   # w1 k1
        load_x(nc.sync, 0, 2)
        load_x(nc.scalar, 0, 3)
        load_w(nc.gpsimd, w1t, w1, 8, nchunk=8)                   # w1 k2-3
        load_x(nc.sync, 0, 4)
        load_x(nc.scalar, 0, 5)
        load_w(nc.gpsimd, w1t, w1, 16, nchunk=8)                  # w1 k4-5
        load_x(nc.sync, 0, 6)
        load_x(nc.scalar, 0, 7)
        load_w(nc.gpsimd, w1t, w1, 24, nchunk=8)                  # w1 k6-7
        nc.gpsimd.dma_start(bt[:], bias[:])
        # x(b1) on the HWDGE queues behind x(b0); w2 streams on gpsimd
        for k in range(8):
            load_x(nc.sync if k % 2 == 0 else nc.scalar, 1, k)
        for qc in range(4):
            load_w(nc.gpsimd, w2t, w2, 8 * qc, nchunk=8)          # w2 qc block

        for k in range(8):
            s1(0, k)
        for k in range(8):
            s1(1, k)
        for qc in range(4):
            s2(0, qc)
        for qc in range(4):
            s2(1, qc)

    nc.compile()
    return nc


def _get_program():
    if "nc" not in _CACHE:
        _CACHE["nc"] = _build_program()
    return _CACHE["nc"]


def _ensure_ntff_hook():
    """Bridge the axon NTFF profile hook when the image's antenv lacks it."""
    import sys, types

    try:
        from antenv.axon_hooks import get_axon_ntff_profile_hook  # noqa: F401

        return
    except ImportError:
        pass
    try:
        from trn_agent_boot.trn_boot import _ntff_profile_via_ctypes

        hook = _ntff_profile_via_ctypes("/opt/axon/libaxon_pjrt.so")
        mod = types.ModuleType("antenv.axon_hooks")
        _h = {"hook": hook}
        mod.set_axon_ntff_profile_hook = lambda h: _h.__setitem__("hook", h)
        mod.get_axon_ntff_profile_hook = lambda: _h["hook"]
        sys.modules["antenv.axon_hooks"] = mod
        import antenv

        antenv.axon_hooks = mod
    except Exception:
        pass


def kernel(x, factorL, factorR, bias):
    global LAST_RESULT
    import ml_dtypes
    from concourse.bass_utils import run_bass_kernel_spmd

    BF = ml_dtypes.bfloat16
    x = np.asarray(x, dtype=np.float32)
    factorL = np.asarray(factorL, dtype=np.float32)
    factorR = np.asarray(factorR, dtype=np.float32)
    bias = np.asarray(bias, dtype=np.float32)

    # ---- host-side marshalling (not device-timed) ----
    xt = np.ascontiguousarray(x.reshape(TOK, 4096).T).astype(BF)  # (4096, 8192)

    # w1 device layout: [pp, (k, pc, qc, j)] = factorL[k, q', pc*128+pp]
    # with q' ordering per k: qperm[q'] = 8*(q'%64) + 2*(q'//128) + (q'%128)//64
    qp = np.arange(512)
    qperm = 8 * (qp % 64) + 2 * (qp // 128) + (qp % 128) // 64
    w1dev = np.empty((128, 16384), dtype=BF)
    for k in range(8):
        Bk = factorL[k][qperm]                       # [q', p]
        arr = Bk.T.reshape(4, 128, 512)              # [pc, pp, q']
        w1dev[:, k * 2048 : (k + 1) * 2048] = (
            arr.transpose(1, 0, 2).reshape(128, 2048).astype(BF)
        )
    w1dev = np.ascontiguousarray(w1dev)

    # w2 device layout: per qc block of 4096 cols (k, sc, j):
    #   partitions 0:64  = factorR[2qc][sc*128+j, 64k+m]   (m = partition)
    #   partitions 64:128= factorR[2qc+1][...]
    w2dev = np.empty((128, 16384), dtype=BF)
    for qc in range(4):
        for half, l in ((0, 2 * qc), (1, 2 * qc + 1)):
            E = factorR[l]                            # [s, r]
            arr = E.reshape(4, 128, 8, 64)            # [sc, j, k, m]
            w2dev[64 * half : 64 * half + 64, qc * 4096 : (qc + 1) * 4096] = (
                arr.transpose(3, 2, 0, 1).reshape(64, 4096).astype(BF)
            )
    w2dev = np.ascontiguousarray(w2dev)

    biasdev = np.ascontiguousarray(
        bias.reshape(4, 128, 8).transpose(1, 2, 0).reshape(128, 32)
    )

    in_maps = []
    for c in range(NCORES):
        xc = xt[:, c * TPC : (c + 1) * TPC]  # (4096 feat, 1024 tok) bf16
        # rows (k,pc,pp) cols (b,t) -> [(b k pp), (pc t)]
        xdev = np.ascontiguousarray(
            xc.reshape(8, 4, 128, NB, T)
            .transpose(3, 0, 2, 1, 4)
            .reshape(NB * 8 * 128, 2048)
        )
        in_maps.append({"x": xdev, "w1": w1dev, "w2": w2dev, "bias": biasdev})

    nc = _get_program()
    trace = os.environ.get("BUTTERFLY_TRACE", "0") == "1"
    if trace:
        _ensure_ntff_hook()
    LAST_RESULT = run_bass_kernel_spmd(
        nc, in_maps, list(range(NCORES)), trace=trace
    )
    # device out rows = (b, l, j), cols = (sc, t)  ->  (tok, feat 8s+l)
    parts = []
    for c in range(NCORES):
        o = np.asarray(LAST_RESULT.results[c]["out"]).astype(np.float32)
        y = o.reshape(NB, 8, 128, 4, T).transpose(0, 4, 3, 2, 1).reshape(TPC, 4096)
        parts.append(y)
    return np.concatenate(parts, axis=0).reshape(4, 2048, 4096)
